# revision 22
# baseline (speedup 1.0000x reference)
"""Trainium2 Bass kernel for nn_CRPSSpectralLoss.

Math (see reference):
  loss = crps_p + 0.1 * crps_f  where each CRPS =
      mean_m |pred - tgt|  (term1)  -  0.5*(1-eps)*mean spread (term2),
  spread = pairwise-L1 over the M=16 ensemble, and crps_f applies the same
  on |rfft2(x)| low-passed to the [kh<32, kw<16] corner.

Kernel strategy (8 cores, data-parallel over B=8 -> 1 sample/core):
  * Pairwise |a-b| sums via the max-trick:
      sum_{ordered pairs}|xi-xj| = 4*sum_{i<j}max(xi,xj) - 2(M-1)*sum_i xi
    -> 15 fused scalar_tensor_tensor(max, accum_out=sum) ops on DVE (fp16,
    2x mode), no separate reduction pass.
  * MAE via |a-b| = 2*max(a,b) - a - b, same fused op with a broadcast target.
  * FFT corner as two small DFT matmuls on the tensor engine:
      M1: psum[w, khblock] = x^T @ [cos|-sin]   (stationary = image, fp16)
      M2: X_re/X_im with the complex combine done by PSUM accumulation.
    Sum_i x_i comes free from the DC coefficient X_re[0,0].
  * Per-core output = raw accumulator tile (128x128 f32); host combines in
    float64 (tiny) and returns the scalar.

Self-contained: hardcodes shapes from the problem spec; imports only
numpy + concourse (on PYTHONPATH in this environment).
"""

import math

import numpy as np

B, M, C, H, W = 8, 16, 3, 128, 128
G = H * W
CUT_H, CUT_W = 32, 16
Gf = H * (W // 2 + 1)
LAMBDA_FREQ = 0.1
EPS = 0.05 / M

# acc column map
COL_PAIR = 1          # 1..15  pointwise pair-max sums (per d)
COL_MAE = 16          # pointwise sum max(x, t)
COL_SPAIR = 17        # 17..31 spectral pair-max sums (partitions 0:16)
COL_SMAE = 32         # 32..34 spectral sum max(|X|, |Xt|), one col per c
COL_S3F = 35          # 2*sum|X|
COL_STF = 36          # 2*sum|Xt|
COL_DC = 64           # 64..112 DC (=sum x) per image, partition 0
COL_DCT = 112         # 112..115 target DC, partition 0


def dft_consts():
    h = np.arange(H)
    kh = np.arange(CUT_H)
    ang_h = 2 * np.pi * np.outer(h, kh) / H
    fh = np.concatenate([np.cos(ang_h), -np.sin(ang_h)], axis=1)  # (128, 64)
    w = np.arange(W)
    kw = np.arange(CUT_W)
    ang_w = 2 * np.pi * np.outer(w, kw) / W
    fw = np.concatenate(
        [np.cos(ang_w), -np.sin(ang_w), np.sin(ang_w)], axis=1
    )  # (128, 48) = [re | im | -im]
    return fh.astype(np.float16), fw.astype(np.float16)


def build_nc():
    """Build the per-core Bass program (same NEFF on all 8 cores)."""
    from contextlib import ExitStack

    from concourse import bacc, bass, mybir, tile

    f32 = mybir.dt.float32
    f16 = mybir.dt.float16

    # Bacc (not raw Bass): its compile() runs generate_event_semaphores /
    # move_matmul_waits_to_ldweights, which split multi-wait instructions
    # down to the 1-wait-per-instruction TRN2 hardware limit.
    nc = bacc.Bacc("TRN2", target_bir_lowering=False, debug=False)

    x_dram = nc.declare_dram_parameter("x", [M, C, H, W], f32, isOutput=False)
    t_dram = nc.declare_dram_parameter("t", [C, H, W], f32, isOutput=False)
    fh_dram = nc.declare_dram_parameter("fh", [H, 2 * CUT_H], f16, isOutput=False)
    fw_dram = nc.declare_dram_parameter("fw", [W, 3 * CUT_W], f16, isOutput=False)
    res_dram = nc.declare_dram_parameter("res", [128, 128], f32, isOutput=True)

    with tile.TileContext(nc) as tc, ExitStack() as ctx:
        pool = ctx.enter_context(tc.tile_pool(name="main", bufs=1))
        ps1 = ctx.enter_context(
            tc.tile_pool(name="ps1", bufs=2, space=bass.MemorySpace.PSUM)
        )
        ps2 = ctx.enter_context(
            tc.tile_pool(name="ps2", bufs=2, space=bass.MemorySpace.PSUM)
        )
        spool = ctx.enter_context(tc.tile_pool(name="scratch", bufs=2))

        # ---- persistent tiles ----
        x_f = pool.tile([128, M, C, W], f32)      # x, partition = h
        t_f = pool.tile([128, C, W], f32)
        x_h = pool.tile([128, M, C, W], f16)
        t_h = pool.tile([128, C, W], f16)
        fh_sb = pool.tile([128, 2 * CUT_H], f16)
        fw_sb = pool.tile([128, 3 * CUT_W], f16)
        y_h = pool.tile([128, M * C + C, 2, CUT_H], f16)   # DFT stage-1 out
        xm = pool.tile([16, C, M, CUT_H], f16)             # |X| magnitudes
        xtm = pool.tile([16, C, CUT_H], f16)               # target |X|
        acc = pool.tile([128, 128], f32)
        pw = pool.tile([128, M, C, W], f16)                # STT out scratch
        sf = pool.tile([16, C, M, CUT_H], f16)             # spectral scratch
        # write-once sink tiles for the cross-engine wait absorbers (each
        # DVE instruction may carry only ONE semaphore wait)
        sink1 = pool.tile([128, 1], f32)
        sink2 = pool.tile([128, 1], f32)
        sink3 = pool.tile([1, 1], f32)
        sink4 = pool.tile([1, 1], f32)
        # per-group DC staging (fresh tiles -> DC copies only wait on PE)
        dcs = [pool.tile([1, 8], f32, name=f"dc{g}", tag=f"dc{g}") for g in range(6)]
        dct = pool.tile([1, C], f32)

        # ---- loads ----
        nc.sync.dma_start(out=x_f[:], in_=x_dram.ap().rearrange("m c h w -> h m c w"))
        nc.sync.dma_start(out=t_f[:], in_=t_dram.ap().rearrange("c h w -> h c w"))
        nc.sync.dma_start(out=fh_sb[:], in_=fh_dram.ap())
        nc.sync.dma_start(out=fw_sb[:], in_=fw_dram.ap())

        nc.vector.memset(acc[:], 0.0)

        # ---- casts (ACT) ----
        nc.scalar.copy(out=x_h[:], in_=x_f[:])
        nc.scalar.copy(out=t_h[:], in_=t_f[:])

        # ---- pointwise CRPS on DVE (fp16, fused max+sum) ----
        # walrus requires <=3D STT operands; all slices here are contiguous,
        # so use flat 2D views.
        # The STT ISA struct has a single sync-wait slot, so absorb the
        # cross-engine (ACT-cast) waits into cheap DVE copies first.
        one = 1.0
        nc.vector.tensor_copy(out=sink1[:], in_=x_h[:, 0, 0, 0:1])
        nc.vector.tensor_copy(out=sink2[:], in_=t_h[:, 0, 0:1])
        x_flat = x_h[:].rearrange("p m c w -> p (m c w)")
        pw_flat = pw[:].rearrange("p m c w -> p (m c w)")
        P = C * W  # 384 elements per member per partition
        for d in range(1, M):
            n = (M - d) * P
            nc.vector.scalar_tensor_tensor(
                out=pw_flat[:, :n],
                in0=x_flat[:, :n],
                scalar=one,
                in1=x_flat[:, d * P :],
                op0=mybir.AluOpType.mult,
                op1=mybir.AluOpType.max,
                accum_out=acc[:, COL_PAIR + d - 1 : COL_PAIR + d],
            )
        nc.vector.scalar_tensor_tensor(
            out=pw[:].rearrange("p m c w -> p m (c w)"),
            in0=x_h[:].rearrange("p m c w -> p m (c w)"),
            scalar=one,
            in1=t_h[:].rearrange("p c w -> p (c w)").unsqueeze(1)
                .broadcast_to((128, M, P)),
            op0=mybir.AluOpType.mult,
            op1=mybir.AluOpType.max,
            accum_out=acc[:, COL_MAE : COL_MAE + 1],
        )

        # ---- FFT stage 1 (PE): psum[w, khblock] = img^T @ fh ----
        # image order s = c*16 + m so that m is contiguous in y_h's free dim
        for g in range(6):
            y_ps = ps1.tile([128, 512], f32, tag="y_ps")
            for k in range(8):
                s = g * 8 + k
                c, m = s // M, s % M
                nc.tensor.matmul(
                    y_ps[:, k * 64 : (k + 1) * 64],
                    x_h[:, m, c, :],
                    fh_sb[:],
                    start=True,
                    stop=True,
                )
            nc.scalar.copy(out=y_h[:, g * 8 : (g + 1) * 8, :, :], in_=y_ps[:])
        y_pst_full = ps1.tile([128, 512], f32, tag="y_ps")
        y_pst = y_pst_full[:, 0:192]
        for c in range(C):
            nc.tensor.matmul(
                y_pst[:, c * 64 : (c + 1) * 64],
                t_h[:, c, :],
                fh_sb[:],
                start=True,
                stop=True,
            )
        nc.scalar.copy(out=y_h[:, M * C : M * C + C, :, :], in_=y_pst[:])

        # ---- FFT stage 2 (PE): complex combine via PSUM accumulation ----
        fwre = fw_sb[:, 0:16]
        fwim = fw_sb[:, 16:32]
        fwimn = fw_sb[:, 32:48]

        def stage2(psx, img_lo, img_n):
            yre = y_h[:, img_lo : img_lo + img_n, 0, :]
            yim = y_h[:, img_lo : img_lo + img_n, 1, :]
            nc.tensor.matmul(psx[:, 0, :, :], fwre, yre, start=True, stop=False)
            nc.tensor.matmul(psx[:, 0, :, :], fwimn, yim, start=False, stop=True)
            nc.tensor.matmul(psx[:, 1, :, :], fwim, yre, start=True, stop=False)
            nc.tensor.matmul(psx[:, 1, :, :], fwre, yim, start=False, stop=True)

        for g in range(6):
            psx = ps2.tile([16, 2, 8, CUT_H], f32, tag="psx")
            stage2(psx, g * 8, 8)
            # DC (= sum of x per image) before any abs; fresh staging tile,
            # on ACT so psx's reader set stays single-semaphore for the PE
            nc.scalar.copy(out=dcs[g][:], in_=psx[0:1, 0, :, 0])
            # |X| = sqrt(re^2 + im^2); per-group tiles avoid slot-reuse
            # deps that would need a second wait on DVE adds
            sq_re = spool.tile([16, 256], f32, name=f"sq_re{g}", tag=f"sq_re{g}")
            sq_im = spool.tile([16, 256], f32, name=f"sq_im{g}", tag=f"sq_im{g}")
            s2 = spool.tile([16, 256], f32, name=f"s2_{g}", tag=f"s2_{g}")
            nc.scalar.square(out=sq_re[:], in_=psx[:, 0, :, :])
            nc.scalar.square(out=sq_im[:], in_=psx[:, 1, :, :])
            nc.vector.tensor_add(s2[:], sq_re[:], sq_im[:])
            c, half = g // 2, g % 2
            nc.scalar.sqrt(
                out=xm[:, c, half * 8 : (half + 1) * 8, :], in_=s2[:]
            )

        psxt_full = ps2.tile([16, 2, 8, CUT_H], f32, tag="psx")
        psxt = psxt_full[:, :, 0:C, :]
        stage2(psxt, M * C, C)
        nc.scalar.copy(out=dct[:], in_=psxt[0:1, 0, :, 0])
        sq_ret = spool.tile([16, 96], f32, tag="sq_ret")
        sq_imt = spool.tile([16, 96], f32, tag="sq_imt")
        s2t = spool.tile([16, 96], f32, tag="s2t")
        nc.scalar.square(out=sq_ret[:], in_=psxt[:, 0, :, :])
        nc.scalar.square(out=sq_imt[:], in_=psxt[:, 1, :, :])
        nc.vector.tensor_add(s2t[:], sq_ret[:], sq_imt[:])
        nc.scalar.sqrt(out=xtm[:], in_=s2t[:])

        # ---- spectral CRPS on DVE (<=3D APs) ----
        # absorb ACT-sqrt waits (STT has one sync-wait slot)
        nc.vector.tensor_copy(out=sink3[:], in_=xm[0:1, 0, 0, 0:1])
        nc.vector.tensor_copy(out=sink4[:], in_=xtm[0:1, 0, 0:1])
        xm3 = xm[:].rearrange("p c m k -> p c (m k)")   # (16, 3, 512)
        sf3 = sf[:].rearrange("p c m k -> p c (m k)")
        for d in range(1, M):
            n = (M - d) * CUT_H
            nc.vector.scalar_tensor_tensor(
                out=sf3[:, :, :n],
                in0=xm3[:, :, :n],
                scalar=one,
                in1=xm3[:, :, d * CUT_H :],
                op0=mybir.AluOpType.mult,
                op1=mybir.AluOpType.max,
                accum_out=acc[0:16, COL_SPAIR + d - 1 : COL_SPAIR + d],
            )
        for c in range(C):
            nc.vector.scalar_tensor_tensor(
                out=sf[:, c],
                in0=xm[:, c],
                scalar=one,
                in1=xtm[:, c].unsqueeze(1).broadcast_to((16, M, CUT_H)),
                op0=mybir.AluOpType.mult,
                op1=mybir.AluOpType.max,
                accum_out=acc[0:16, COL_SMAE + c : COL_SMAE + c + 1],
            )
        xm2 = xm[:].rearrange("p c m k -> p (c m k)")
        sf2 = sf[:].rearrange("p c m k -> p (c m k)")
        nc.vector.scalar_tensor_tensor(
            out=sf2,
            in0=xm2,
            scalar=one,
            in1=xm2,
            op0=mybir.AluOpType.mult,
            op1=mybir.AluOpType.add,
            accum_out=acc[0:16, COL_S3F : COL_S3F + 1],
        )
        sft = spool.tile([16, C, CUT_H], f16, tag="sft")
        nc.vector.scalar_tensor_tensor(
            out=sft[:].rearrange("p c k -> p (c k)"),
            in0=xtm[:].rearrange("p c k -> p (c k)"),
            scalar=one,
            in1=xtm[:].rearrange("p c k -> p (c k)"),
            op0=mybir.AluOpType.mult,
            op1=mybir.AluOpType.add,
            accum_out=acc[0:16, COL_STF : COL_STF + 1],
        )

        # ---- output: acc plus the DC staging tiles (separate small DMAs
        # so no engine ever needs a second wait slot) ----
        nc.sync.dma_start(out=res_dram.ap()[:, 0:COL_DC], in_=acc[:, 0:COL_DC])
        for g in range(6):
            nc.sync.dma_start(
                out=res_dram.ap()[0:1, COL_DC + g * 8 : COL_DC + (g + 1) * 8],
                in_=dcs[g][:],
            )
        nc.sync.dma_start(
            out=res_dram.ap()[0:1, COL_DCT : COL_DCT + C], in_=dct[:]
        )

    nc.compile()
    return nc


_NC_CACHE = None


def _get_nc():
    global _NC_CACHE
    if _NC_CACHE is None:
        _NC_CACHE = build_nc()
    return _NC_CACHE


def combine_results(res_list):
    """res_list: per-core (128, 128) f32 accumulator tiles -> scalar loss."""
    tot = dict(S_pairmax=0.0, S_maxt=0.0, S3=0.0, S_t=0.0,
               Sf_pairmax=0.0, Sf_maxt=0.0, S3f=0.0, S_tf=0.0)
    for res in res_list:
        r = res.astype(np.float64)
        tot['S_pairmax'] += r[:, COL_PAIR : COL_PAIR + 15].sum()
        tot['S_maxt'] += r[:, COL_MAE].sum()
        tot['S3'] += r[0, COL_DC : COL_DC + M * C].sum()
        tot['S_t'] += r[0, COL_DCT : COL_DCT + C].sum()
        tot['Sf_pairmax'] += r[0:16, COL_SPAIR : COL_SPAIR + 15].sum()
        tot['Sf_maxt'] += r[0:16, COL_SMAE : COL_SMAE + 3].sum()
        tot['S3f'] += r[0:16, COL_S3F].sum() / 2.0
        tot['S_tf'] += r[0:16, COL_STF].sum() / 2.0

    P_pt = C * G
    mae_pt = 2 * tot['S_maxt'] - tot['S3'] - M * tot['S_t']
    spread_pt = 4 * tot['S_pairmax'] - 2 * (M - 1) * tot['S3']
    term1_p = mae_pt / (B * M * P_pt)
    term2_p = spread_pt / ((M - 1) * B * M * P_pt) * (1 - EPS)
    crps_p = term1_p - 0.5 * term2_p

    P_f = C * Gf
    mae_f = 2 * tot['Sf_maxt'] - tot['S3f'] - M * tot['S_tf']
    spread_f = 4 * tot['Sf_pairmax'] - 2 * (M - 1) * tot['S3f']
    term1_f = mae_f / (B * M * P_f)
    term2_f = spread_f / ((M - 1) * B * M * P_f) * (1 - EPS)
    crps_f = term1_f - 0.5 * term2_f

    return np.float32(crps_p + LAMBDA_FREQ * crps_f)


def make_in_maps(target, output):
    fh, fw = dft_consts()
    target = np.ascontiguousarray(np.asarray(target, dtype=np.float32))
    output = np.ascontiguousarray(np.asarray(output, dtype=np.float32))
    return [
        {"x": output[b], "t": target[b], "fh": fh, "fw": fw}
        for b in range(B)
    ]


def kernel(target, output):
    from concourse.bass_utils import run_bass_kernel_spmd

    nc = _get_nc()
    in_maps = make_in_maps(target, output)
    results = run_bass_kernel_spmd(nc, in_maps, list(range(B))).results
    return combine_results([results[b]["res"] for b in range(B)])


# revision 36
# speedup vs baseline: 1.7609x; 1.7609x over previous
"""Trainium2 Bass kernel for nn_CRPSSpectralLoss (v2).

Math (see reference.py):
  loss = crps_p + 0.1 * crps_f, each CRPS = mean|pred-tgt| - 0.5*(1-eps)*spread,
  spread = pairwise L1 over the M=16 ensemble; crps_f applies the same on
  |rfft2(x)| low-passed to the [kh<32, kw<16] corner.

Kernel strategy (8 cores, data-parallel over B; 1 sample per core):
  * max-trick: sum_{ordered pairs}|xi-xj| = 4*sum_{i<j}max(xi,xj) - 2(M-1)*sum xi
    and |a-b| = 2*max(a,b) - a - b, so ALL elementwise work is tensor_tensor
    max in fp16 (DVE 2x_1p mode), written to scratch.
  * Scratch is reduced with ones-stationary matmuls on the tensor engine,
    accumulating into PSUM regions (partition-dim contraction).
  * FFT corner via two DFT matmul stages:
      M1 (per image): psum[w, khblock] = img^T @ [cos|-sin]  (stationary=img)
      M2: column-strip tiling (tile_position=(0,32*khq)) puts the (kw, khq)
      grid on 128 PSUM partitions; complex combine by PSUM accumulation.
    |X| then needs one square/add/sqrt chain over a single (128, 408) tile.
    The target is carried as ensemble member 16 (17-member pair batches).
    sum(x) falls out of the DC coefficient (partition 0, khsub 0).
  * Per-core output = raw reduction rows; host combines in float64.

Self-contained: hardcodes the problem shapes; imports numpy + concourse only.
"""

import numpy as np

B, M, C, H, W = 8, 16, 3, 128, 128
G = H * W
CUT_H, CUT_W = 32, 16
Gf = H * (W // 2 + 1)
LAMBDA_FREQ = 0.1
EPS = 0.05 / M

MT = M + 1  # ensemble members + target

# fin/res column offsets (all on partition 0)
OFF_PAIR = 0        # [0:512)    pointwise pair-max partial sums
OFF_MAE = 512       # [512:1024) pointwise max(x,t) partial sums
OFF_SPAIR = 1024    # [1024:1384) spectral pair-max partials (360)
OFF_STMAX = 1384    # [1384:1408) spectral max(|X|,|Xt|) partials (24)
OFF_S3F = 1408      # [1408:1792) sum|X| partials (384)
OFF_STF = 1792      # [1792:1816) sum|Xt| partials (24)
OFF_DC = 1816       # [1816:1867) DC per image: (c, m) with m=16 -> target
RES_W = OFF_DC + 51  # 1867; fully written


def dft_consts():
    h = np.arange(H)
    kh = np.arange(CUT_H)
    ang_h = 2 * np.pi * np.outer(h, kh) / H
    fh = np.concatenate([np.cos(ang_h), -np.sin(ang_h)], axis=1)  # (128, 64)
    w = np.arange(W)
    kw = np.arange(CUT_W)
    ang_w = 2 * np.pi * np.outer(w, kw) / W
    fw = np.concatenate(
        [np.cos(ang_w), -np.sin(ang_w), np.sin(ang_w)], axis=1
    )  # (128, 48) = [re | im | -im]
    return fh.astype(np.float16), fw.astype(np.float16)


def build_nc():
    from contextlib import ExitStack

    from concourse import bacc, bass, mybir, tile

    f32 = mybir.dt.float32
    f16 = mybir.dt.float16
    MAX = mybir.AluOpType.max

    nc = bacc.Bacc("TRN2", target_bir_lowering=False, debug=False)

    x_dram = nc.declare_dram_parameter("x", [M, C, H, W], f32, isOutput=False)
    t_dram = nc.declare_dram_parameter("t", [C, H, W], f32, isOutput=False)
    fh_dram = nc.declare_dram_parameter("fh", [H, 2 * CUT_H], f16, isOutput=False)
    fw_dram = nc.declare_dram_parameter("fw", [W, 3 * CUT_W], f16, isOutput=False)
    res_dram = nc.declare_dram_parameter("res", [1, RES_W], f32, isOutput=True)

    with tile.TileContext(nc) as tc, ExitStack() as ctx:
        pool = ctx.enter_context(tc.tile_pool(name="main", bufs=1))
        pwp = ctx.enter_context(tc.tile_pool(name="pwp", bufs=6))
        ps1 = ctx.enter_context(
            tc.tile_pool(name="ps1", bufs=2, space=bass.MemorySpace.PSUM)
        )
        psf = ctx.enter_context(
            tc.tile_pool(name="psf", bufs=1, space=bass.MemorySpace.PSUM)
        )

        # ---- persistent tiles ----
        x_f = pool.tile([128, M, C, W], f32)
        t_f = pool.tile([128, C, W], f32)
        x_h = pool.tile([128, M, C, W], f16)
        t_h = pool.tile([128, C, W], f16)
        fh_sb = pool.tile([128, 2 * CUT_H], f16)
        fw_sb = pool.tile([128, 3 * CUT_W], f16)
        ones = pool.tile([128, 1], f16)
        y_h = pool.tile([128, M * C + C, 2, CUT_H], f16)
        xm = pool.tile([128, C, MT, 8], f16)    # |X|: part=(khq,kw), m16=target
        # per-d spectral scratch (cheap; avoids WAR serialization against the
        # end-of-kernel spectral reductions)
        sfs = [pool.tile([128, C, MT, 8], f16, name=f"sf{d}", tag=f"sf{d}")
               for d in range(1, MT)]
        sq_re = pool.tile([128, C * MT * 8], f32)
        sq_im = pool.tile([128, C * MT * 8], f32)
        s2 = pool.tile([128, C * MT * 8], f32)
        dc_all = pool.tile([1, C, MT], f32)
        fin = pool.tile([1, RES_W], f32)

        # PSUM accumulators (partition 0 rows)
        ps_pair = psf.tile([1, 512], f32, tag="ps_pair")
        ps_mae = psf.tile([1, 512], f32, tag="ps_mae")
        ps_m1 = psf.tile([1, 512], f32, tag="ps_m1")   # spair(360) + stmax(24)
        ps_m2 = psf.tile([1, 512], f32, tag="ps_m2")   # s3f(384) + stf(24)
        # M2 PSUM (one bank each, all four kh-quarter strips); allocate as
        # full 512-col banks so the 408-col view stays inside one bank
        psum_re_f = psf.tile([128, 512], f32, tag="psum_re")
        psum_im_f = psf.tile([128, 512], f32, tag="psum_im")
        psum_re = psum_re_f[:, 0 : C * MT * 8].rearrange(
            "p (c m k) -> p c m k", c=C, m=MT)
        psum_im = psum_im_f[:, 0 : C * MT * 8].rearrange(
            "p (c m k) -> p c m k", c=C, m=MT)

        # ---- loads: x split by channel on three queues (parallel DMA) ----
        xr = x_dram.ap().rearrange("m c h w -> h m c w")
        nc.sync.dma_start(out=x_f[:, :, 0, :], in_=xr[:, :, 0, :])
        nc.scalar.dma_start(out=x_f[:, :, 1, :], in_=xr[:, :, 1, :])
        nc.sync.dma_start(out=x_f[:, :, 2, :], in_=xr[:, :, 2, :])
        nc.scalar.dma_start(out=t_f[:], in_=t_dram.ap().rearrange("c h w -> h c w"))
        nc.sync.dma_start(out=fh_sb[:], in_=fh_dram.ap())
        nc.sync.dma_start(out=fw_sb[:], in_=fw_dram.ap())
        nc.gpsimd.memset(ones[:], 1.0)

        # junk-row hygiene for the strip-tiled M2 PSUM (rows 32q+16..32q+32
        # are never written by matmuls; zero so |X| rows are benign zeros)
        nc.vector.memset(psum_re[:], 0.0)
        nc.vector.memset(psum_im[:], 0.0)

        # ---- casts (ACT), per channel so FFT/pairwise start early ----
        for c in range(C):
            nc.scalar.copy(out=x_h[:, :, c, :], in_=x_f[:, :, c, :])
        nc.scalar.copy(out=t_h[:], in_=t_f[:])

        # ================= pointwise CRPS =================
        x2 = x_h[:].rearrange("p m c w -> p (m c w)")
        P = C * W
        pw_tiles = []
        for d in range(1, M):
            n = (M - d) * P
            pw = pwp.tile([128, 15 * P], f16, name=f"pw{d}", tag="pw")
            nc.vector.tensor_tensor(out=pw[:, :n], in0=x2[:, :n],
                                    in1=x2[:, d * P :], op=MAX)
            pw_tiles.append((pw, n))
        mae_t = []
        for c in range(C):
            mw = pwp.tile([128, M * W], f16, name=f"mw{c}", tag="mw")
            nc.vector.tensor_tensor(
                out=mw[:].rearrange("p (m w) -> p m w", m=M),
                in0=x_h[:, :, c, :],
                in1=t_h[:, c, :].unsqueeze(1).broadcast_to((128, M, W)),
                op=MAX)
            mae_t.append(mw)

        # ================= FFT stage 1 (PE) =================
        for g in range(6):
            y_ps = ps1.tile([128, 512], f32, tag="y_ps")
            for k in range(8):
                s = g * 8 + k
                c, m = s // M, s % M
                nc.tensor.matmul(y_ps[:, k * 64 : (k + 1) * 64],
                                 x_h[:, m, c, :], fh_sb[:],
                                 start=True, stop=True)
            nc.scalar.copy(out=y_h[:, g * 8 : (g + 1) * 8, :, :], in_=y_ps[:])
        y_pst = ps1.tile([128, 512], f32, tag="y_ps")
        for c in range(C):
            nc.tensor.matmul(y_pst[:, c * 64 : (c + 1) * 64],
                             t_h[:, c, :], fh_sb[:], start=True, stop=True)
        nc.scalar.copy(out=y_h[:, M * C : M * C + C, :, :], in_=y_pst[:, 0:192])

        # ================= FFT stage 2 (PE, column strips) =================
        fwre, fwim, fwimn = fw_sb[:, 0:16], fw_sb[:, 16:32], fw_sb[:, 32:48]
        for q in range(4):
            tp = (0, 32 * q)
            lo, hi = 32 * q, 32 * q + 16
            khs = slice(q * 8, (q + 1) * 8)
            for c in range(C):
                yre = y_h[:, c * M : (c + 1) * M, 0, khs]
                yim = y_h[:, c * M : (c + 1) * M, 1, khs]
                out_re = psum_re[lo:hi, c, 0:M, :].rearrange("p m k -> p (m k)")
                out_im = psum_im[lo:hi, c, 0:M, :].rearrange("p m k -> p (m k)")
                nc.tensor.matmul(out_re, fwre, yre,
                                 start=True, stop=False, tile_position=tp)
                nc.tensor.matmul(out_re, fwimn, yim,
                                 start=False, stop=True, tile_position=tp)
                nc.tensor.matmul(out_im, fwim, yre,
                                 start=True, stop=False, tile_position=tp)
                nc.tensor.matmul(out_im, fwre, yim,
                                 start=False, stop=True, tile_position=tp)
            # target -> member slot 16 (per channel: psum free dims are
            # only contiguous within one (c, m) block)
            for c in range(C):
                ytre = y_h[:, M * C + c, 0, khs]
                ytim = y_h[:, M * C + c, 1, khs]
                nc.tensor.matmul(psum_re[lo:hi, c, M, :], fwre, ytre,
                                 start=True, stop=False, tile_position=tp)
                nc.tensor.matmul(psum_re[lo:hi, c, M, :], fwimn, ytim,
                                 start=False, stop=True, tile_position=tp)
                nc.tensor.matmul(psum_im[lo:hi, c, M, :], fwim, ytre,
                                 start=True, stop=False, tile_position=tp)
                nc.tensor.matmul(psum_im[lo:hi, c, M, :], fwre, ytim,
                                 start=False, stop=True, tile_position=tp)

        # DC (= sum over pixels) per image: strip q=0, kw=0, khsub=0
        nc.scalar.copy(out=dc_all[:], in_=psum_re[0:1, :, :, 0])

        # |X| = sqrt(re^2+im^2) on the whole (128, 408) block at once
        nc.scalar.square(out=sq_re[:], in_=psum_re[:].rearrange("p c m k -> p (c m k)"))
        nc.scalar.square(out=sq_im[:], in_=psum_im[:].rearrange("p c m k -> p (c m k)"))
        nc.gpsimd.tensor_add(s2[:], sq_re[:], sq_im[:])
        nc.scalar.sqrt(out=xm[:].rearrange("p c m k -> p (c m k)"), in_=s2[:])

        # ================= spectral CRPS (17 members, target last) ==========
        for d in range(1, MT):
            sf = sfs[d - 1]
            nc.vector.tensor_tensor(
                out=sf[:, :, : MT - d, :].rearrange("p c m k -> p c (m k)"),
                in0=xm[:, :, : MT - d, :].rearrange("p c m k -> p c (m k)"),
                in1=xm[:, :, d:, :].rearrange("p c m k -> p c (m k)"),
                op=MAX)

        # ================= pointwise reductions (PE, ones-stationary) =======
        chunks = []
        for pw, n in pw_tiles:
            for off in range(0, n, 512):
                chunks.append((pw, off, min(512, n - off)))
        for i, (pw, off, w) in enumerate(chunks):
            nc.tensor.matmul(ps_pair[:, :w], ones[:], pw[:, off : off + w],
                             start=(i == 0), stop=(i == len(chunks) - 1))
        mchunks = [(mw, off) for mw in mae_t for off in range(0, M * W, 512)]
        for i, (mw, off) in enumerate(mchunks):
            nc.tensor.matmul(ps_mae[:, 0:512], ones[:], mw[:, off : off + 512],
                             start=(i == 0), stop=(i == len(mchunks) - 1))

        # ================= spectral reductions (PE, last) =================
        # two sequential accumulation groups (a PSUM bank allows only one
        # open group at a time): x-pairs first, then the target pairs
        for d in range(1, M):
            sf = sfs[d - 1]
            nc.tensor.matmul(
                ps_m1[:, : (M - d) * 24], ones[:],
                sf[:, :, : M - d, :].rearrange("p c m k -> p c (m k)"),
                start=(d == 1), stop=(d == 15))
        for d in range(1, MT):
            sf = sfs[d - 1]
            nc.tensor.matmul(ps_m1[:, 360:384], ones[:], sf[:, :, M - d, :],
                             start=(d == 1), stop=(d == MT - 1))
        nc.tensor.matmul(ps_m2[:, 0:384], ones[:],
                         xm[:, :, 0:M, :].rearrange("p c m k -> p c (m k)"),
                         start=True, stop=True)
        nc.tensor.matmul(ps_m2[:, 384:408], ones[:], xm[:, :, M, :],
                         start=True, stop=True)

        # ================= pack + output =================
        nc.scalar.copy(out=fin[:, OFF_PAIR : OFF_PAIR + 512], in_=ps_pair[:])
        nc.scalar.copy(out=fin[:, OFF_MAE : OFF_MAE + 512], in_=ps_mae[:])
        nc.scalar.copy(out=fin[:, OFF_SPAIR : OFF_SPAIR + 384], in_=ps_m1[:, 0:384])
        nc.scalar.copy(out=fin[:, OFF_S3F : OFF_S3F + 408], in_=ps_m2[:, 0:408])
        nc.scalar.copy(out=fin[:, OFF_DC : OFF_DC + C * MT],
                       in_=dc_all[:].rearrange("p c m -> p (c m)"))
        nc.sync.dma_start(out=res_dram.ap(), in_=fin[:])

    nc.compile()
    return nc


_NC_CACHE = None


def _get_nc():
    global _NC_CACHE
    if _NC_CACHE is None:
        _NC_CACHE = build_nc()
    return _NC_CACHE


def sums_from_res(res):
    """Raw per-sample sums from one core's res row (float64)."""
    r = np.asarray(res, dtype=np.float64).reshape(-1)
    dc = r[OFF_DC : OFF_DC + C * MT].reshape(C, MT)
    return dict(
        S_pairmax=r[OFF_PAIR : OFF_PAIR + 512].sum(),
        S_maxt=r[OFF_MAE : OFF_MAE + 512].sum(),
        S3=dc[:, 0:M].sum(),
        S_t=dc[:, M].sum(),
        Sf_pairmax=r[OFF_SPAIR : OFF_SPAIR + 360].sum(),
        Sf_maxt=r[OFF_STMAX : OFF_STMAX + 24].sum(),
        S3f=r[OFF_S3F : OFF_S3F + 384].sum(),
        S_tf=r[OFF_STF : OFF_STF + 24].sum(),
    )


def combine_sums(sums_list):
    tot = {k: sum(s[k] for s in sums_list) for k in sums_list[0]}
    P_pt = C * G
    mae_pt = 2 * tot['S_maxt'] - tot['S3'] - M * tot['S_t']
    spread_pt = 4 * tot['S_pairmax'] - 2 * (M - 1) * tot['S3']
    term1_p = mae_pt / (B * M * P_pt)
    term2_p = spread_pt / ((M - 1) * B * M * P_pt) * (1 - EPS)
    crps_p = term1_p - 0.5 * term2_p

    P_f = C * Gf
    mae_f = 2 * tot['Sf_maxt'] - tot['S3f'] - M * tot['S_tf']
    spread_f = 4 * tot['Sf_pairmax'] - 2 * (M - 1) * tot['S3f']
    term1_f = mae_f / (B * M * P_f)
    term2_f = spread_f / ((M - 1) * B * M * P_f) * (1 - EPS)
    crps_f = term1_f - 0.5 * term2_f

    return np.float32(crps_p + LAMBDA_FREQ * crps_f)


def combine_results(res_list):
    return combine_sums([sums_from_res(res) for res in res_list])


def make_in_maps(target, output):
    fh, fw = dft_consts()
    target = np.ascontiguousarray(np.asarray(target, dtype=np.float32))
    output = np.ascontiguousarray(np.asarray(output, dtype=np.float32))
    return [
        {"x": output[b], "t": target[b], "fh": fh, "fw": fw}
        for b in range(B)
    ]


def kernel(target, output):
    from concourse.bass_utils import run_bass_kernel_spmd

    nc = _get_nc()
    in_maps = make_in_maps(target, output)
    results = run_bass_kernel_spmd(nc, in_maps, list(range(B))).results
    return combine_results([results[b]["res"] for b in range(B)])
